# revision 24
# baseline (speedup 1.0000x reference)
"""Trainium2 Bass kernel for Luong 'general' attention scoring.

reference:
    proj     = einsum('sbh,kh->sbk', enc, W) + b          # [S,B,H]
    energies = einsum('bh,sbh->bs', hidden[0], proj)      # [B,S]
    out      = softmax(energies, -1)[:, None, :]          # [B,1,S]

Math reduction used here:
    energies[b,s] = hidden[b] . (W @ enc[s,b]) + hidden[b] . b_attn
                  = (W^T @ hidden[b]) . enc[s,b] + c_b
c_b is constant over s, so softmax is invariant to it -> b_attn drops out
entirely and the per-(s,b) work is a single H-length dot product against
q[b] = W^T @ hidden[b].  That turns the problem memory-bound: the cost is
streaming encoder_outputs (256 MB) once.

Sharding: data-parallel over batch. B=16 across 8 cores -> 2 batches/core.
Each core gets enc[:, 2i:2i+2, :] (32 MB, contiguous slice), the full W
(4 MB, replicated) and its hidden slice pre-transposed to [K,2] layout.

Per-core kernel (v5):
  prologue    : W chunks head the sync DMA ring; the enc stream is gated
                behind them (real semaphore dep) so W gets full bandwidth.
                bf16 PE warm-up matmuls trip the HAM clock gate cheaply
                (f32 matmuls lower to 2 HW passes - warmups must be bf16).
                16 f32 matmuls compute q[2,1024], broadcast to
                qbp[128,2048] PSUM via 4 one-hot matmuls (read by DVE
                multiplies through the PSUM port) and copied once to SBUF
                (read by GpSimd multiplies via the DVE<->GpSimd shared
                port, which the DVE then never touches).
  main loop   : 32 s-tiles, ~2.9us/tile DMA-bound. enc tile [128,2048]
                (1 MB contiguous DMA). Multiply on DVE (3 of 4 tiles, in1
                PSUM) or GpSimd (every 4th tile, in1 SBUF, ~4.5us).
                Reduces to et_all[:, b, t]: b=0 ACT activation(Copy,
                accum_out); b=1 alternates DVE reduce_sum / ACT.
                No per-tile transpose or copy.
  epilogue    : ONE PE transpose et_all[128, 64] -> [64,128] PSUM
                (partition = b*32+t, free = s%128).  All softmax ops then
                run 64-partition-wide: reduce_max [64,1], cross-partition
                max/sum folded via tiny PE transposes + one-hot-group
                matmul broadcasts, ACT Exp(bias, accum_out) at 0.6us,
                DVE scale, strided DMA out.
  (tensor_tensor_reduce would fuse mul+reduce but crashes the exec unit
  on this HW/ucode combo - see probe3/probe4.)
"""

import numpy as np

S = 4096
B = 16
H = 1024
N_CORES = 8
B_LOC = B // N_CORES          # 2
P = 128
NT = S // P                   # 32 s-tiles
KC = H // P                   # 8 k-chunks
FREE = B_LOC * H              # 2048
NR = B_LOC * NT               # 64 rows of the transposed energies

_cache = {}


def _build_nc():
    import concourse.bass as bass
    import concourse.tile as tile
    from concourse import bacc, mybir
    from concourse.masks import make_identity
    from concourse.tile_rust import add_dep_helper

    f32 = mybir.dt.float32
    bf16 = mybir.dt.bfloat16
    # Bacc (not plain Bass): its compile() pass splits multi-sem waits on
    # matmuls; plain Bass dies in walrus with "Too many sync wait commands".
    nc = bacc.Bacc("TRN2")

    enc = nc.dram_tensor("enc", [S, FREE], f32, kind="ExternalInput")
    w = nc.dram_tensor("w", [H, H], f32, kind="ExternalInput")
    ht = nc.dram_tensor("ht", [P, KC * B_LOC], f32, kind="ExternalInput")
    sel = nc.dram_tensor("sel", [B_LOC, B_LOC * P], f32, kind="ExternalInput")
    grp = nc.dram_tensor("grp", [B_LOC, NR], f32, kind="ExternalInput")
    out = nc.dram_tensor("out", [B_LOC, S], f32, kind="ExternalOutput")

    with tile.TileContext(nc) as tc:
        with (
            tc.tile_pool(name="singles", bufs=1) as singles,
            tc.tile_pool(name="encpool", bufs=8) as encpool,
            tc.tile_pool(name="tmppool", bufs=3) as tmppool,
            tc.tile_pool(name="tmp2pool", bufs=1) as tmp2pool,
        ):
            ident = singles.tile([P, P], f32)
            make_identity(nc, ident)
            wub = singles.tile([P, P], bf16)
            nc.gpsimd.memset(wub, 1.0)

            # host consts: one-hot row selectors (partition-offset memsets
            # trip the start-partition check, so these come from DRAM)
            onehot = singles.tile([B_LOC, B_LOC, P], f32)
            nc.scalar.dma_start(
                out=onehot, in_=sel.rearrange("p (b m) -> p b m", b=B_LOC)
            )
            # group selector: grp[b, r] = 1 iff r // NT == b
            grp_sb = singles.tile([B_LOC, NR], f32)
            nc.scalar.dma_start(out=grp_sb, in_=grp[:, :])

            # hidden^T on the scalar ring (parallel with W on sync ring)
            ht_sb = singles.tile([P, KC, B_LOC], f32)
            nc.scalar.dma_start(
                out=ht_sb, in_=ht.rearrange("p (c b) -> p c b", b=B_LOC)
            )

            # tiny dummy DMA first: absorbs the ~8us first-transfer
            # latency of the sync ring so W chunk 0 lands promptly
            dummy = singles.tile([1, 64], f32)
            nc.sync.dma_start(out=dummy, in_=w[0:1, 0:64])

            # W chunks at the HEAD of the sync ring
            w_all = singles.tile([P, KC, H], f32)
            w_dmas = []
            for c in range(KC):
                w_dmas.append(
                    nc.sync.dma_start(
                        out=w_all[:, c, :], in_=w[c * P : (c + 1) * P, :]
                    )
                )

            q_sb = singles.tile([B_LOC, H], f32)
            q_bcast_sb = singles.tile([P, FREE], f32)
            et_all = singles.tile([P, B_LOC, NT], f32)

            # q_bcast in PSUM: DVE multiplies read it through DVE's PSUM
            # port, leaving the shared DVE<->GpSimd SBUF port to GpSimd.
            # Its pool closes before the epilogue to free PSUM banks.
            qbpsum_cm = tc.tile_pool(name="qbpsum", bufs=1, space="PSUM")
            qbpsum = qbpsum_cm.__enter__()
            qbp = qbpsum.tile([P, FREE], f32)

            with tc.tile_pool(name="psA", bufs=1, space="PSUM") as psA:
                # bf16 PE warm-up (single-pass) to trip HAM before the
                # 2-pass f32 q matmuls. Transpose-mode doesn't count for
                # HAM, so real MATMULs.
                wu = psA.tile([P, P], f32)
                for _ in range(36):
                    nc.tensor.matmul(wu, wub, wub, start=True, stop=True)

                # q[b, h] = sum_k hidden[b, k] * W[k, h]  -> PSUM [2, 1024]
                qp = psA.tile([B_LOC, H], f32)
                for c in range(KC):
                    for ns in range(H // 512):
                        nc.tensor.matmul(
                            qp[:, ns * 512 : (ns + 1) * 512],
                            ht_sb[:, c, :],
                            w_all[:, c, ns * 512 : (ns + 1) * 512],
                            start=(c == 0),
                            stop=(c == KC - 1),
                        )
                nc.scalar.copy(out=q_sb, in_=qp)

                # broadcast q over 128 partitions into PSUM
                for b in range(B_LOC):
                    for ns in range(H // 512):
                        nc.tensor.matmul(
                            qbp[:, b * H + ns * 512 : b * H + (ns + 1) * 512],
                            onehot[:, b, :],
                            q_sb[:, ns * 512 : (ns + 1) * 512],
                            start=True,
                            stop=True,
                        )
                # SBUF copy of the broadcast for the GpSimd multiplies
                nc.scalar.copy(out=q_bcast_sb, in_=qbp)

            tmp2 = tmp2pool.tile([P, FREE], f32)
            for t in range(NT):
                enc_t = encpool.tile([P, FREE], f32, tag="enc")
                enc_dma = nc.sync.dma_start(
                    out=enc_t, in_=enc[t * P : (t + 1) * P, :]
                )
                if t == 0:
                    # gate the enc stream behind W's completion so the W
                    # transfers get full HBM bandwidth
                    add_dep_helper(
                        enc_dma.ins,
                        w_dmas[-1].ins,
                        sync=True,
                        reason="enc stream after W chunks",
                    )
                tmp = tmppool.tile([P, FREE], f32, tag="tmp")
                if t % 8 in (2, 5, 7):
                    nc.gpsimd.tensor_mul(out=tmp, in0=enc_t, in1=q_bcast_sb)
                else:
                    nc.vector.tensor_mul(out=tmp, in0=enc_t, in1=qbp)
                nc.scalar.activation(
                    out=tmp2[:, 0:H],
                    in_=tmp[:, 0:H],
                    func=mybir.ActivationFunctionType.Copy,
                    accum_out=et_all[:, 0, t : t + 1],
                )
                if t % 2 == 0:
                    nc.vector.reduce_sum(
                        et_all[:, 1, t : t + 1], tmp[:, H:FREE], axis=mybir.AxisListType.X
                    )
                else:
                    nc.scalar.activation(
                        out=tmp2[:, H:FREE],
                        in_=tmp[:, H:FREE],
                        func=mybir.ActivationFunctionType.Copy,
                        accum_out=et_all[:, 1, t : t + 1],
                    )

            qbpsum_cm.__exit__(None, None, None)

            # ---- epilogue: single transpose + 64-partition-wide softmax ----
            with tc.tile_pool(name="psB", bufs=1, space="PSUM") as psB:
                eT_ps = psB.tile([NR, P], f32)   # [64, 128]: row b*32+t
                nc.tensor.transpose(
                    eT_ps, et_all.rearrange("p b t -> p (b t)"), ident
                )
                eT = singles.tile([NR, P], f32)
                nc.scalar.copy(out=eT, in_=eT_ps)

                # global max per b: row max, transpose, max over rows
                m64 = singles.tile([NR, 1], f32)
                nc.vector.reduce_max(m64, eT, axis=mybir.AxisListType.X)
                m64T_ps = psB.tile([1, NR], f32)
                nc.tensor.transpose(m64T_ps, m64, ident[0:NR, 0:NR])
                m64T = singles.tile([1, B_LOC, NT], f32)
                nc.vector.tensor_copy(m64T, m64T_ps.rearrange("o (b t) -> o b t", b=B_LOC))
                negm2 = singles.tile([1, B_LOC], f32)
                nc.vector.tensor_reduce(
                    negm2, m64T, axis=mybir.AxisListType.X,
                    op=mybir.AluOpType.max, negate=True,
                )
                # broadcast -max[b] to its 32 rows: bias64 = grp.T @ negm2.T
                negm2T_ps = psB.tile([B_LOC, 1], f32)
                nc.tensor.transpose(negm2T_ps, negm2, ident[0:1, 0:1])
                negm2T = singles.tile([B_LOC, 1], f32)
                nc.scalar.copy(out=negm2T, in_=negm2T_ps)
                bias64_ps = psB.tile([NR, 1], f32)
                nc.tensor.matmul(
                    bias64_ps, grp_sb, negm2T, start=True, stop=True
                )
                bias64 = singles.tile([NR, 1], f32)
                nc.scalar.copy(out=bias64, in_=bias64_ps)

                # exp + per-row sums, then per-b Z over rows
                p64 = singles.tile([NR, P], f32)
                z64 = singles.tile([NR, 1], f32)
                nc.scalar.activation(
                    out=p64,
                    in_=eT,
                    func=mybir.ActivationFunctionType.Exp,
                    bias=bias64,
                    scale=1.0,
                    accum_out=z64,
                )
                z64T_ps = psB.tile([1, NR], f32)
                nc.tensor.transpose(z64T_ps, z64, ident[0:NR, 0:NR])
                z64T = singles.tile([1, B_LOC, NT], f32)
                nc.vector.tensor_copy(z64T, z64T_ps.rearrange("o (b t) -> o b t", b=B_LOC))
                z2 = singles.tile([1, B_LOC], f32)
                nc.vector.reduce_sum(z2, z64T, axis=mybir.AxisListType.X)
                rz2 = singles.tile([1, B_LOC], f32)
                nc.vector.reciprocal(rz2, z2)
                rz2T_ps = psB.tile([B_LOC, 1], f32)
                nc.tensor.transpose(rz2T_ps, rz2, ident[0:1, 0:1])
                rz2T = singles.tile([B_LOC, 1], f32)
                nc.scalar.copy(out=rz2T, in_=rz2T_ps)
                rz64_ps = psB.tile([NR, 1], f32)
                nc.tensor.matmul(rz64_ps, grp_sb, rz2T, start=True, stop=True)
                rz64 = singles.tile([NR, 1], f32)
                nc.scalar.copy(out=rz64, in_=rz64_ps)

                nc.vector.tensor_scalar_mul(out=p64, in0=p64, scalar1=rz64)
                nc.sync.dma_start(
                    out=out.rearrange("b (t j) -> (b t) j", j=P), in_=p64
                )

    nc.finalize()
    return nc


def get_nc():
    if "nc" not in _cache:
        _cache["nc"] = _build_nc()
    return _cache["nc"]


def make_in_maps(hidden, encoder_outputs, W_attn):
    """Shard full inputs into per-core input maps."""
    w_full = np.ascontiguousarray(W_attn, dtype=np.float32)
    sel = np.zeros((B_LOC, B_LOC, P), dtype=np.float32)
    for b in range(B_LOC):
        sel[b, b, :] = 1.0
    sel = sel.reshape(B_LOC, B_LOC * P)
    grp = np.zeros((B_LOC, B_LOC, NT), dtype=np.float32)
    for b in range(B_LOC):
        grp[b, b, :] = 1.0
    grp = grp.reshape(B_LOC, NR)
    in_maps = []
    for i in range(N_CORES):
        b0 = i * B_LOC
        enc_i = np.ascontiguousarray(
            encoder_outputs[:, b0 : b0 + B_LOC, :], dtype=np.float32
        ).reshape(S, FREE)
        # ht[p, c*B_LOC + b] = hidden[0, b0+b, c*128+p]
        h_i = np.ascontiguousarray(hidden[0, b0 : b0 + B_LOC, :], dtype=np.float32)
        ht_i = np.ascontiguousarray(
            h_i.reshape(B_LOC, KC, P).transpose(2, 1, 0).reshape(P, KC * B_LOC)
        )
        in_maps.append(
            {"enc": enc_i, "w": w_full, "ht": ht_i, "sel": sel, "grp": grp}
        )
    return in_maps


def kernel(hidden, encoder_outputs, W_attn, b_attn, **run_kwargs):
    """Full inputs in, full output out.  b_attn is mathematically irrelevant
    (constant shift per softmax row) and is ignored."""
    from concourse.bass_utils import run_bass_kernel_spmd

    nc = get_nc()
    in_maps = make_in_maps(hidden, encoder_outputs, W_attn)
    res = run_bass_kernel_spmd(
        nc, in_maps, core_ids=list(range(N_CORES)), **run_kwargs
    )
    out = np.empty((B, 1, S), dtype=np.float32)
    for i in range(N_CORES):
        out[i * B_LOC : (i + 1) * B_LOC, 0, :] = res.results[i]["out"]
    _cache["last_result"] = res
    return out


# revision 25
# speedup vs baseline: 1.0162x; 1.0162x over previous
"""Trainium2 Bass kernel for Luong 'general' attention scoring.

reference:
    proj     = einsum('sbh,kh->sbk', enc, W) + b          # [S,B,H]
    energies = einsum('bh,sbh->bs', hidden[0], proj)      # [B,S]
    out      = softmax(energies, -1)[:, None, :]          # [B,1,S]

Math reduction used here:
    energies[b,s] = hidden[b] . (W @ enc[s,b]) + hidden[b] . b_attn
                  = (W^T @ hidden[b]) . enc[s,b] + c_b
c_b is constant over s, so softmax is invariant to it -> b_attn drops out
entirely and the per-(s,b) work is a single H-length dot product against
q[b] = W^T @ hidden[b].  That turns the problem memory-bound: the cost is
streaming encoder_outputs (256 MB) once.

Sharding: data-parallel over batch. B=16 across 8 cores -> 2 batches/core.
Each core gets enc[:, 2i:2i+2, :] (32 MB, contiguous slice), the full W
(4 MB, replicated) and its hidden slice pre-transposed to [K,2] layout.

Per-core kernel (v5):
  prologue    : W chunks head the sync DMA ring; the enc stream is gated
                behind them (real semaphore dep) so W gets full bandwidth.
                bf16 PE warm-up matmuls trip the HAM clock gate cheaply
                (f32 matmuls lower to 2 HW passes - warmups must be bf16).
                16 f32 matmuls compute q[2,1024], broadcast to
                qbp[128,2048] PSUM via 4 one-hot matmuls (read by DVE
                multiplies through the PSUM port) and copied once to SBUF
                (read by GpSimd multiplies via the DVE<->GpSimd shared
                port, which the DVE then never touches).
  main loop   : 32 s-tiles, ~2.9us/tile DMA-bound. enc tile [128,2048]
                (1 MB contiguous DMA). Multiply on DVE (3 of 4 tiles, in1
                PSUM) or GpSimd (every 4th tile, in1 SBUF, ~4.5us).
                Reduces to et_all[:, b, t]: b=0 ACT activation(Copy,
                accum_out); b=1 alternates DVE reduce_sum / ACT.
                No per-tile transpose or copy.
  epilogue    : ONE PE transpose et_all[128, 64] -> [64,128] PSUM
                (partition = b*32+t, free = s%128).  All softmax ops then
                run 64-partition-wide: reduce_max [64,1], cross-partition
                max/sum folded via tiny PE transposes + one-hot-group
                matmul broadcasts, ACT Exp(bias, accum_out) at 0.6us,
                DVE scale, strided DMA out.
  (tensor_tensor_reduce would fuse mul+reduce but crashes the exec unit
  on this HW/ucode combo - see probe3/probe4.)
"""

import numpy as np

S = 4096
B = 16
H = 1024
N_CORES = 8
B_LOC = B // N_CORES          # 2
P = 128
NT = S // P                   # 32 s-tiles
KC = H // P                   # 8 k-chunks
FREE = B_LOC * H              # 2048
NR = B_LOC * NT               # 64 rows of the transposed energies

_cache = {}


def _build_nc():
    import concourse.bass as bass
    import concourse.tile as tile
    from concourse import bacc, mybir
    from concourse.masks import make_identity
    from concourse.tile_rust import add_dep_helper

    f32 = mybir.dt.float32
    bf16 = mybir.dt.bfloat16
    # Bacc (not plain Bass): its compile() pass splits multi-sem waits on
    # matmuls; plain Bass dies in walrus with "Too many sync wait commands".
    nc = bacc.Bacc("TRN2")

    enc = nc.dram_tensor("enc", [S, FREE], f32, kind="ExternalInput")
    w = nc.dram_tensor("w", [H, H], f32, kind="ExternalInput")
    ht = nc.dram_tensor("ht", [P, KC * B_LOC], f32, kind="ExternalInput")
    sel = nc.dram_tensor("sel", [B_LOC, B_LOC * P], f32, kind="ExternalInput")
    grp = nc.dram_tensor("grp", [B_LOC, NR], f32, kind="ExternalInput")
    out = nc.dram_tensor("out", [B_LOC, S], f32, kind="ExternalOutput")

    with tile.TileContext(nc) as tc:
        with (
            tc.tile_pool(name="singles", bufs=1) as singles,
            tc.tile_pool(name="encpool", bufs=8) as encpool,
            tc.tile_pool(name="tmppool", bufs=3) as tmppool,
            tc.tile_pool(name="tmp2pool", bufs=1) as tmp2pool,
        ):
            ident = singles.tile([P, P], f32)
            make_identity(nc, ident)
            wub = singles.tile([P, P], bf16)
            nc.gpsimd.memset(wub, 1.0)

            # host consts: one-hot row selectors (partition-offset memsets
            # trip the start-partition check, so these come from DRAM)
            onehot = singles.tile([B_LOC, B_LOC, P], f32)
            nc.scalar.dma_start(
                out=onehot, in_=sel.rearrange("p (b m) -> p b m", b=B_LOC)
            )
            # group selector: grp[b, r] = 1 iff r // NT == b
            grp_sb = singles.tile([B_LOC, NR], f32)
            nc.scalar.dma_start(out=grp_sb, in_=grp[:, :])

            # hidden^T on the scalar ring (parallel with W on sync ring)
            ht_sb = singles.tile([P, KC, B_LOC], f32)
            nc.scalar.dma_start(
                out=ht_sb, in_=ht.rearrange("p (c b) -> p c b", b=B_LOC)
            )

            # tiny dummy DMA first: absorbs the ~8us first-transfer
            # latency of the sync ring so W chunk 0 lands promptly
            dummy = singles.tile([1, 64], f32)
            nc.sync.dma_start(out=dummy, in_=w[0:1, 0:64])

            # W chunks at the HEAD of the sync ring
            w_all = singles.tile([P, KC, H], f32)
            w_dmas = []
            for c in range(KC):
                w_dmas.append(
                    nc.sync.dma_start(
                        out=w_all[:, c, :], in_=w[c * P : (c + 1) * P, :]
                    )
                )

            q_sb = singles.tile([B_LOC, H], f32)
            q_bcast_sb = singles.tile([P, FREE], f32)
            et_all = singles.tile([P, B_LOC, NT], f32)

            # q_bcast in PSUM: DVE multiplies read it through DVE's PSUM
            # port, leaving the shared DVE<->GpSimd SBUF port to GpSimd.
            # Its pool closes before the epilogue to free PSUM banks.
            qbpsum_cm = tc.tile_pool(name="qbpsum", bufs=1, space="PSUM")
            qbpsum = qbpsum_cm.__enter__()
            qbp = qbpsum.tile([P, FREE], f32)

            with tc.tile_pool(name="psA", bufs=1, space="PSUM") as psA:
                # bf16 PE warm-up (single-pass) to trip HAM before the
                # 2-pass f32 q matmuls. Transpose-mode doesn't count for
                # HAM, so real MATMULs.
                wu = psA.tile([P, P], f32)
                for _ in range(24):
                    nc.tensor.matmul(wu, wub, wub, start=True, stop=True)

                # q[b, h] = sum_k hidden[b, k] * W[k, h]  -> PSUM [2, 1024]
                qp = psA.tile([B_LOC, H], f32)
                for c in range(KC):
                    for ns in range(H // 512):
                        nc.tensor.matmul(
                            qp[:, ns * 512 : (ns + 1) * 512],
                            ht_sb[:, c, :],
                            w_all[:, c, ns * 512 : (ns + 1) * 512],
                            start=(c == 0),
                            stop=(c == KC - 1),
                        )
                nc.scalar.copy(out=q_sb, in_=qp)

                # broadcast q over 128 partitions into PSUM
                for b in range(B_LOC):
                    for ns in range(H // 512):
                        nc.tensor.matmul(
                            qbp[:, b * H + ns * 512 : b * H + (ns + 1) * 512],
                            onehot[:, b, :],
                            q_sb[:, ns * 512 : (ns + 1) * 512],
                            start=True,
                            stop=True,
                        )
                # SBUF copy of the broadcast for the GpSimd multiplies
                nc.scalar.copy(out=q_bcast_sb, in_=qbp)

            tmp2 = tmp2pool.tile([P, FREE], f32)
            for t in range(NT):
                enc_t = encpool.tile([P, FREE], f32, tag="enc")
                enc_dma = nc.sync.dma_start(
                    out=enc_t, in_=enc[t * P : (t + 1) * P, :]
                )
                if t == 0:
                    # gate the enc stream behind W's completion so the W
                    # transfers get full HBM bandwidth
                    add_dep_helper(
                        enc_dma.ins,
                        w_dmas[-1].ins,
                        sync=True,
                        reason="enc stream after W chunks",
                    )
                tmp = tmppool.tile([P, FREE], f32, tag="tmp")
                if t % 4 == 3:
                    nc.gpsimd.tensor_mul(out=tmp, in0=enc_t, in1=q_bcast_sb)
                else:
                    nc.vector.tensor_mul(out=tmp, in0=enc_t, in1=qbp)
                nc.scalar.activation(
                    out=tmp2[:, 0:H],
                    in_=tmp[:, 0:H],
                    func=mybir.ActivationFunctionType.Copy,
                    accum_out=et_all[:, 0, t : t + 1],
                )
                if t % 2 == 0:
                    nc.vector.reduce_sum(
                        et_all[:, 1, t : t + 1], tmp[:, H:FREE], axis=mybir.AxisListType.X
                    )
                else:
                    nc.scalar.activation(
                        out=tmp2[:, H:FREE],
                        in_=tmp[:, H:FREE],
                        func=mybir.ActivationFunctionType.Copy,
                        accum_out=et_all[:, 1, t : t + 1],
                    )

            qbpsum_cm.__exit__(None, None, None)

            # ---- epilogue: single transpose + 64-partition-wide softmax ----
            with tc.tile_pool(name="psB", bufs=1, space="PSUM") as psB:
                eT_ps = psB.tile([NR, P], f32)   # [64, 128]: row b*32+t
                nc.tensor.transpose(
                    eT_ps, et_all.rearrange("p b t -> p (b t)"), ident
                )
                eT = singles.tile([NR, P], f32)
                nc.scalar.copy(out=eT, in_=eT_ps)

                # global max per b: row max, transpose, max over rows
                m64 = singles.tile([NR, 1], f32)
                nc.vector.reduce_max(m64, eT, axis=mybir.AxisListType.X)
                m64T_ps = psB.tile([1, NR], f32)
                nc.tensor.transpose(m64T_ps, m64, ident[0:NR, 0:NR])
                m64T = singles.tile([1, B_LOC, NT], f32)
                nc.vector.tensor_copy(m64T, m64T_ps.rearrange("o (b t) -> o b t", b=B_LOC))
                negm2 = singles.tile([1, B_LOC], f32)
                nc.vector.tensor_reduce(
                    negm2, m64T, axis=mybir.AxisListType.X,
                    op=mybir.AluOpType.max, negate=True,
                )
                # broadcast -max[b] to its 32 rows: bias64 = grp.T @ negm2.T
                negm2T_ps = psB.tile([B_LOC, 1], f32)
                nc.tensor.transpose(negm2T_ps, negm2, ident[0:1, 0:1])
                negm2T = singles.tile([B_LOC, 1], f32)
                nc.scalar.copy(out=negm2T, in_=negm2T_ps)
                bias64_ps = psB.tile([NR, 1], f32)
                nc.tensor.matmul(
                    bias64_ps, grp_sb, negm2T, start=True, stop=True
                )
                bias64 = singles.tile([NR, 1], f32)
                nc.scalar.copy(out=bias64, in_=bias64_ps)

                # exp + per-row sums, then per-b Z over rows
                p64 = singles.tile([NR, P], f32)
                z64 = singles.tile([NR, 1], f32)
                nc.scalar.activation(
                    out=p64,
                    in_=eT,
                    func=mybir.ActivationFunctionType.Exp,
                    bias=bias64,
                    scale=1.0,
                    accum_out=z64,
                )
                z64T_ps = psB.tile([1, NR], f32)
                nc.tensor.transpose(z64T_ps, z64, ident[0:NR, 0:NR])
                z64T = singles.tile([1, B_LOC, NT], f32)
                nc.vector.tensor_copy(z64T, z64T_ps.rearrange("o (b t) -> o b t", b=B_LOC))
                z2 = singles.tile([1, B_LOC], f32)
                nc.vector.reduce_sum(z2, z64T, axis=mybir.AxisListType.X)
                rz2 = singles.tile([1, B_LOC], f32)
                nc.vector.reciprocal(rz2, z2)
                rz2T_ps = psB.tile([B_LOC, 1], f32)
                nc.tensor.transpose(rz2T_ps, rz2, ident[0:1, 0:1])
                rz2T = singles.tile([B_LOC, 1], f32)
                nc.scalar.copy(out=rz2T, in_=rz2T_ps)
                rz64_ps = psB.tile([NR, 1], f32)
                nc.tensor.matmul(rz64_ps, grp_sb, rz2T, start=True, stop=True)
                rz64 = singles.tile([NR, 1], f32)
                nc.scalar.copy(out=rz64, in_=rz64_ps)

                nc.vector.tensor_scalar_mul(out=p64, in0=p64, scalar1=rz64)
                nc.sync.dma_start(
                    out=out.rearrange("b (t j) -> (b t) j", j=P), in_=p64
                )

    nc.finalize()
    return nc


def get_nc():
    if "nc" not in _cache:
        _cache["nc"] = _build_nc()
    return _cache["nc"]


def make_in_maps(hidden, encoder_outputs, W_attn):
    """Shard full inputs into per-core input maps."""
    w_full = np.ascontiguousarray(W_attn, dtype=np.float32)
    sel = np.zeros((B_LOC, B_LOC, P), dtype=np.float32)
    for b in range(B_LOC):
        sel[b, b, :] = 1.0
    sel = sel.reshape(B_LOC, B_LOC * P)
    grp = np.zeros((B_LOC, B_LOC, NT), dtype=np.float32)
    for b in range(B_LOC):
        grp[b, b, :] = 1.0
    grp = grp.reshape(B_LOC, NR)
    in_maps = []
    for i in range(N_CORES):
        b0 = i * B_LOC
        enc_i = np.ascontiguousarray(
            encoder_outputs[:, b0 : b0 + B_LOC, :], dtype=np.float32
        ).reshape(S, FREE)
        # ht[p, c*B_LOC + b] = hidden[0, b0+b, c*128+p]
        h_i = np.ascontiguousarray(hidden[0, b0 : b0 + B_LOC, :], dtype=np.float32)
        ht_i = np.ascontiguousarray(
            h_i.reshape(B_LOC, KC, P).transpose(2, 1, 0).reshape(P, KC * B_LOC)
        )
        in_maps.append(
            {"enc": enc_i, "w": w_full, "ht": ht_i, "sel": sel, "grp": grp}
        )
    return in_maps


def kernel(hidden, encoder_outputs, W_attn, b_attn, **run_kwargs):
    """Full inputs in, full output out.  b_attn is mathematically irrelevant
    (constant shift per softmax row) and is ignored."""
    from concourse.bass_utils import run_bass_kernel_spmd

    nc = get_nc()
    in_maps = make_in_maps(hidden, encoder_outputs, W_attn)
    res = run_bass_kernel_spmd(
        nc, in_maps, core_ids=list(range(N_CORES)), **run_kwargs
    )
    out = np.empty((B, 1, S), dtype=np.float32)
    for i in range(N_CORES):
        out[i * B_LOC : (i + 1) * B_LOC, 0, :] = res.results[i]["out"]
    _cache["last_result"] = res
    return out
